# revision 20
# baseline (speedup 1.0000x reference)
"""Trainium2 Bass kernel: MeanHinAggregator (GNN message passing).

Reference computation (per batch-head element bh):
    z_r  = mean_n(x_neigh_r[bh, n, :]) @ w_neigh_r          (r = 0, 1)
    out  = relu(concat(x_self[bh] @ w_self, (z0 + z1) / 2) + b)

Strategy (pure data parallel over 8 NeuronCores, batch axis sharded):
  * Per core: B_shard=128, H=10 -> 1280 bh rows, mapped partition-major
    (bh = 10*p + g, p = SBUF partition, g = group 0..9) so per-partition
    DRAM runs are contiguous across adjacent groups: x_self loads in ONE
    dma (5 KiB/partition) and the output stores in three batched dmas.
  * The binding constraint is AGGREGATE HBM bandwidth: 8 cores x 44 MB of
    fp32 input against a ~3.25 TB/s ceiling.  A single core streams at
    ~430 GB/s; 8x430 oversubscribes the ceiling, so the arbiter squeezes
    1-2 unlucky cores per run and those define the max-across-cores time
    (self-pacing below fair share was tried and does NOT prevent this; it
    only adds time).  Even groups load whole bh rows (16 KiB packets);
    the last group loads as two halves (8 KiB packets) so the fold can
    start on the first half, shortening the pipeline-drain tail.
  * All neighbour loads ride ONE ring (sync), strictly alternating
    xn0/xn1: DMA engines drain whole DMAs in global enqueue order, so
    in-FIFO alternation is the only way to guarantee the two tensors
    stream together.  Consts (packed into a single dma) + x_self are
    enqueued at t~0 on the scalar ring so they are served inside the
    first neighbour DMAs; stores also ride the scalar ring, gated by
    relu semaphores.
  * Mean over 32 neighbours: in-place strided adds on the Vector engine.
    The first add reads fp32 and writes bf16; the rest run in bf16 which
    gets the DVE 2x packed-16-bit mode.  The x_self projection runs ahead
    of the neighbour stream (it has no neighbour dependency), keeping the
    drain-tail chain to: last half-fold -> transpose -> copy -> two
    matmuls -> relu -> small store.
  * The folded [bh, f] slices are transposed into [f, bh] PSUM layout with
    single-pass bf16 matmuls against a bf16 identity.  Projection matmuls,
    also bf16, use host-precast weights with the 1/(N*NR) mean scale
    folded in.  Bias is added with a K=1 matmul; final ReLU emits fp32.
"""

import numpy as np
import ml_dtypes

import concourse.bacc as bacc
import concourse.bass as bass
import concourse.tile as tile
from concourse import bass_utils, mybir
from concourse._compat import with_exitstack

B, H, N, F = 1024, 10, 32, 128
HALF = 128
D = 2 * HALF
NR = 2
NCORES = 8
BSH = B // NCORES        # 128 batch rows per core
BH = BSH * H             # 1280 bh rows per core
NG = 10                  # groups per core (128 bh rows each)
GF = N * F               # 4096 elements per bh row
HG = GF // 2
F32 = mybir.dt.float32
BF16 = mybir.dt.bfloat16
NPBF16 = ml_dtypes.bfloat16
PRE = 4                  # groups of neighbour tiles in flight


@with_exitstack
def _tile_kernel(ctx, tc, outs, ins):
    nc = tc.nc
    xn0, xn1, xs, wpack, bvec = ins
    (out_d,) = outs

    const = ctx.enter_context(tc.tile_pool(name="const", bufs=1))
    xpool = ctx.enter_context(tc.tile_pool(name="xp", bufs=PRE))
    fpool = ctx.enter_context(tc.tile_pool(name="fp", bufs=3))
    spool = ctx.enter_context(tc.tile_pool(name="sp", bufs=3))
    ppool = ctx.enter_context(tc.tile_pool(name="ps", bufs=2, space="PSUM"))
    pout = ctx.enter_context(tc.tile_pool(name="po", bufs=2, space="PSUM"))

    # Consts + x_self ride the scalar ring, enqueued by the ACT engine's
    # first instructions (t~0) so they are served within the first one or
    # two neighbour DMAs of the sync ring.  This keeps the neighbour
    # stream's first packet off the critical path of the const loads.
    # ident/wS/w0/w1 arrive packed in one [128, 512] tensor (single dma);
    # they are not needed until the first transpose matmul (~13us in).
    xs_all = const.tile([128, NG * F], F32, tag="xs_all")
    nc.scalar.dma_start(xs_all[:], xs[:])
    wp_t = const.tile([128, 4 * 128], BF16, tag="wp")
    nc.scalar.dma_start(wp_t[:], wpack[:])
    ident = wp_t[:, 0:128]
    wS_t = wp_t[:, 128:256]
    w0_t = wp_t[:, 256:384]
    w1_t = wp_t[:, 384:512]
    b_t = const.tile([1, D], BF16, tag="b")
    nc.scalar.dma_start(b_t[:], bvec[:])
    ones_t = const.tile([1, 128], BF16, tag="ones")
    nc.vector.memset(ones_t[:], 1.0)
    # Cast x_self on gpsimd (otherwise idle): a DVE-resident cast would
    # sit ahead of every fold in the DVE program and stall them all until
    # the xs dma lands.
    xs_bf = const.tile([128, NG * F], BF16, tag="xs_bf")
    nc.gpsimd.tensor_copy(xs_bf[:], xs_all[:])

    obuf = const.tile([128, NG * D], F32, tag="obuf")

    def issue_group(g):
        # Both tensors of a group share ONE tile: [t0 rows | t1 rows], so
        # the fold levels fuse into single DVE ops over a [p, 2, x] view.
        t = xpool.tile([128, 2 * GF], F32, tag="t01")
        if g < NG - 1:
            nc.sync.dma_start(t[:, 0:GF], xn0[:, g * GF:(g + 1) * GF])
            nc.sync.dma_start(t[:, GF:2 * GF], xn1[:, g * GF:(g + 1) * GF])
        else:
            # Last group loads as halves so the fold overlaps the final
            # DMAs, shrinking the pipeline-drain tail.
            nc.sync.dma_start(t[:, 0:HG], xn0[:, g * GF:g * GF + HG])
            nc.sync.dma_start(t[:, GF:GF + HG], xn1[:, g * GF:g * GF + HG])
            nc.sync.dma_start(t[:, HG:GF], xn0[:, g * GF + HG:(g + 1) * GF])
            nc.sync.dma_start(t[:, GF + HG:2 * GF],
                              xn1[:, g * GF + HG:(g + 1) * GF])
        return t

    def fold_fused(t):
        """Fold both tensors' 32 slices with ONE DVE op per level, using
        [p, 2, x] strided views over the shared tile (same cycles as two
        ops -- DVE cost is max AP free size -- but half the instruction
        count and per-op overhead).  Sums land at fb[:, 0:F] (xn0) and
        fb[:, 16F:16F+F] (xn1)."""
        fb = fpool.tile([128, 2 * 16 * F], BF16, tag="fb")
        tv = t[:].rearrange("p (t x) -> p t x", t=2)
        fv = fb[:].rearrange("p (t x) -> p t x", t=2)
        nc.vector.tensor_add(fv[:, :, :], tv[:, :, 0:16 * F],
                             tv[:, :, 16 * F:32 * F])
        for lv in (8, 4, 2, 1):
            nc.vector.tensor_add(fv[:, :, 0:lv * F], fv[:, :, 0:lv * F],
                                 fv[:, :, lv * F:2 * lv * F])
        return fb

    def fold_split(t):
        """Last-group reduction: per-tensor, half-at-a-time, each half
        folding its 16 slices pairwise as soon as its dma lands (the
        fused form would wait for the final half of BOTH tensors)."""
        fb = fpool.tile([128, 2 * 16 * F], BF16, tag="fb")
        for off in (0, GF):
            fo = off * 16 * F // GF  # 0 or 16*F: this tensor's fb half
            nc.vector.tensor_add(fb[:, fo:fo + 8 * F], t[:, off:off + 8 * F],
                                 t[:, off + 8 * F:off + 16 * F])
            nc.vector.tensor_add(fb[:, fo + 8 * F:fo + 16 * F],
                                 t[:, off + HG:off + HG + 8 * F],
                                 t[:, off + HG + 8 * F:off + HG + 16 * F])
            nc.vector.tensor_add(fb[:, fo:fo + 8 * F], fb[:, fo:fo + 8 * F],
                                 fb[:, fo + 8 * F:fo + 16 * F])
            for lv in (4, 2, 1):
                nc.vector.tensor_add(fb[:, fo:fo + lv * F],
                                     fb[:, fo:fo + lv * F],
                                     fb[:, fo + lv * F:fo + 2 * lv * F])
        return fb

    def self_part(g):
        """x_self projection for group g: independent of the neighbour
        stream, so it runs ahead and stays off the drain-tail chain."""
        pc = ppool.tile([128, 128], F32, tag="pacc_s")
        nc.tensor.matmul(pc[:], xs_bf[:, g * F:(g + 1) * F], ident,
                         start=True, stop=True)
        sx = spool.tile([128, 128], BF16, tag="sacc_s")
        nc.scalar.activation(sx[:], pc[:], mybir.ActivationFunctionType.Copy)
        po = pout.tile([128, D], F32, tag="po")
        nc.tensor.matmul(po[:, 0:HALF], sx[:], wS_t, start=True, stop=False)
        nc.tensor.matmul(po[:, 0:HALF], ones_t[:], b_t[:, 0:HALF],
                         start=False, stop=True)
        return po

    def compute_group(g, po, fb):
        pacc = ppool.tile([128, 2 * 128], F32, tag="pacc")
        nc.tensor.matmul(pacc[:, 0:128], fb[:, 0:F], ident,
                         start=True, stop=True)
        nc.tensor.matmul(pacc[:, 128:256], fb[:, 16 * F:16 * F + F], ident,
                         start=True, stop=True)

        sacc = spool.tile([128, 2 * 128], BF16, tag="sacc")
        nc.scalar.activation(sacc[:], pacc[:], mybir.ActivationFunctionType.Copy)

        nc.tensor.matmul(po[:, HALF:D], sacc[:, 0:128], w0_t,
                         start=True, stop=False)
        nc.tensor.matmul(po[:, HALF:D], sacc[:, 128:256], w1_t,
                         start=False, stop=False)
        nc.tensor.matmul(po[:, HALF:D], ones_t[:], b_t[:, HALF:D],
                         start=False, stop=True)

        nc.scalar.activation(obuf[:, g * D:(g + 1) * D], po[:],
                             mybir.ActivationFunctionType.Relu)
        # Batched stores on the otherwise-idle scalar ring, enqueued right
        # after the relu that gates them so they fire mid-stream.
        if g == 4:
            nc.scalar.dma_start(out_d[:, 0:5 * D], obuf[:, 0:5 * D])
        elif g == 8:
            nc.scalar.dma_start(out_d[:, 5 * D:9 * D], obuf[:, 5 * D:9 * D])
        elif g == 9:
            nc.scalar.dma_start(out_d[:, 9 * D:NG * D], obuf[:, 9 * D:NG * D])

    pending = [issue_group(g) for g in range(PRE)]
    for g in range(NG):
        t = pending.pop(0)
        if g + PRE < NG:
            pending.append(issue_group(g + PRE))
        po = self_part(g)
        fb = fold_fused(t) if g < NG - 1 else fold_split(t)
        compute_group(g, po, fb)


def build_nc():
    nc = bacc.Bacc("TRN2", target_bir_lowering=False, debug=False)
    # bh rows are partition-major: dram row index = 10*p + g.
    xn0 = nc.dram_tensor("xn0", [128, NG * GF], F32, kind="ExternalInput")
    xn1 = nc.dram_tensor("xn1", [128, NG * GF], F32, kind="ExternalInput")
    xs = nc.dram_tensor("xs", [128, NG * F], F32, kind="ExternalInput")
    wpack = nc.dram_tensor("wpack", [128, 4 * 128], BF16, kind="ExternalInput")
    bvec = nc.dram_tensor("bvec", [1, D], BF16, kind="ExternalInput")
    out = nc.dram_tensor("out", [128, NG * D], F32, kind="ExternalOutput")

    ins = [t.ap() for t in (xn0, xn1, xs, wpack, bvec)]
    with tile.TileContext(nc) as tc:
        _tile_kernel(tc, [out.ap()], ins)
    nc.compile()
    return nc


def make_in_maps(x_self, x_neigh_0, x_neigh_1, w_self, w_neigh_0, w_neigh_1, b):
    """Shard full inputs into per-core input maps (batch axis, 8 ways)."""
    x_self = np.ascontiguousarray(np.asarray(x_self, dtype=np.float32))
    x_neigh_0 = np.ascontiguousarray(np.asarray(x_neigh_0, dtype=np.float32))
    x_neigh_1 = np.ascontiguousarray(np.asarray(x_neigh_1, dtype=np.float32))
    scale = np.float32(1.0 / (N * NR))
    ident = np.eye(128, dtype=np.float32)
    wpack = np.concatenate([
        ident,
        np.asarray(w_self, dtype=np.float32),
        np.asarray(w_neigh_0, dtype=np.float32) * scale,
        np.asarray(w_neigh_1, dtype=np.float32) * scale,
    ], axis=1).astype(NPBF16)
    bvec = np.asarray(b, dtype=np.float32).reshape(1, D).astype(NPBF16)

    in_maps = []
    for c in range(NCORES):
        bs = slice(c * BSH, (c + 1) * BSH)
        in_maps.append({
            "xn0": np.ascontiguousarray(x_neigh_0[bs].reshape(128, NG * GF)),
            "xn1": np.ascontiguousarray(x_neigh_1[bs].reshape(128, NG * GF)),
            "xs": np.ascontiguousarray(x_self[bs].reshape(128, NG * F)),
            "wpack": np.ascontiguousarray(wpack), "bvec": bvec,
        })
    return in_maps


_NC_CACHE = None


def kernel(x_self, x_neigh_0, x_neigh_1, w_self, w_neigh_0, w_neigh_1, b):
    global _NC_CACHE
    if _NC_CACHE is None:
        _NC_CACHE = build_nc()
    in_maps = make_in_maps(x_self, x_neigh_0, x_neigh_1,
                           w_self, w_neigh_0, w_neigh_1, b)
    res = bass_utils.run_bass_kernel_spmd(
        _NC_CACHE, in_maps, core_ids=list(range(NCORES)))
    out = np.concatenate([r["out"].reshape(BH, D) for r in res.results], axis=0)
    return out.reshape(B, H, D)


# revision 21
# speedup vs baseline: 1.0075x; 1.0075x over previous
"""Trainium2 Bass kernel: MeanHinAggregator (GNN message passing).

Reference computation (per batch-head element bh):
    z_r  = mean_n(x_neigh_r[bh, n, :]) @ w_neigh_r          (r = 0, 1)
    out  = relu(concat(x_self[bh] @ w_self, (z0 + z1) / 2) + b)

Strategy (pure data parallel over 8 NeuronCores, batch axis sharded):
  * Per core: B_shard=128, H=10 -> 1280 bh rows, mapped partition-major
    (bh = 10*p + g, p = SBUF partition, g = group 0..9) so per-partition
    DRAM runs are contiguous across adjacent groups: x_self loads in ONE
    dma (5 KiB/partition) and the output stores in three batched dmas.
  * The binding constraint is AGGREGATE HBM bandwidth: 8 cores x 44 MB of
    fp32 input against a ~3.25 TB/s ceiling.  A single core streams at
    ~430 GB/s; 8x430 oversubscribes the ceiling, so the arbiter squeezes
    1-2 unlucky cores per run and those define the max-across-cores time
    (self-pacing below fair share was tried and does NOT prevent this; it
    only adds time).  Even groups load whole bh rows (16 KiB packets);
    the last group loads as two halves (8 KiB packets) so the fold can
    start on the first half, shortening the pipeline-drain tail.
  * All neighbour loads ride ONE ring (sync), strictly alternating
    xn0/xn1: DMA engines drain whole DMAs in global enqueue order, so
    in-FIFO alternation is the only way to guarantee the two tensors
    stream together.  Consts (packed into a single dma) + x_self are
    enqueued at t~0 on the scalar ring so they are served inside the
    first neighbour DMAs; stores also ride the scalar ring, gated by
    relu semaphores.
  * Mean over 32 neighbours: in-place strided adds on the Vector engine.
    The first add reads fp32 and writes bf16; the rest run in bf16 which
    gets the DVE 2x packed-16-bit mode.  The x_self projection runs ahead
    of the neighbour stream (it has no neighbour dependency), keeping the
    drain-tail chain to: last half-fold -> transpose -> copy -> two
    matmuls -> relu -> small store.
  * The folded [bh, f] slices are transposed into [f, bh] PSUM layout with
    single-pass bf16 matmuls against a bf16 identity.  Projection matmuls,
    also bf16, use host-precast weights with the 1/(N*NR) mean scale
    folded in.  Bias is added with a K=1 matmul; final ReLU emits fp32.
"""

import numpy as np
import ml_dtypes

import concourse.bacc as bacc
import concourse.bass as bass
import concourse.tile as tile
from concourse import bass_utils, mybir
from concourse._compat import with_exitstack

B, H, N, F = 1024, 10, 32, 128
HALF = 128
D = 2 * HALF
NR = 2
NCORES = 8
BSH = B // NCORES        # 128 batch rows per core
BH = BSH * H             # 1280 bh rows per core
NG = 10                  # groups per core (128 bh rows each)
GF = N * F               # 4096 elements per bh row
HG = GF // 2
F32 = mybir.dt.float32
BF16 = mybir.dt.bfloat16
NPBF16 = ml_dtypes.bfloat16
PRE = 4                  # groups of neighbour tiles in flight


@with_exitstack
def _tile_kernel(ctx, tc, outs, ins):
    nc = tc.nc
    xn0, xn1, xs, wpack, bvec = ins
    (out_d,) = outs

    const = ctx.enter_context(tc.tile_pool(name="const", bufs=1))
    xpool = ctx.enter_context(tc.tile_pool(name="xp", bufs=PRE))
    fpool = ctx.enter_context(tc.tile_pool(name="fp", bufs=3))
    spool = ctx.enter_context(tc.tile_pool(name="sp", bufs=3))
    ppool = ctx.enter_context(tc.tile_pool(name="ps", bufs=2, space="PSUM"))
    pout = ctx.enter_context(tc.tile_pool(name="po", bufs=2, space="PSUM"))

    # Consts + x_self ride the scalar ring, enqueued by the ACT engine's
    # first instructions (t~0) so they are served within the first one or
    # two neighbour DMAs of the sync ring.  This keeps the neighbour
    # stream's first packet off the critical path of the const loads.
    # ident/wS/w0/w1 arrive packed in one [128, 512] tensor (single dma);
    # they are not needed until the first transpose matmul (~13us in).
    xs_all = const.tile([128, NG * F], F32, tag="xs_all")
    nc.scalar.dma_start(xs_all[:], xs[:])
    wp_t = const.tile([128, 4 * 128], BF16, tag="wp")
    nc.scalar.dma_start(wp_t[:], wpack[:])
    ident = wp_t[:, 0:128]
    wS_t = wp_t[:, 128:256]
    w0_t = wp_t[:, 256:384]
    w1_t = wp_t[:, 384:512]
    b_t = const.tile([1, D], BF16, tag="b")
    nc.scalar.dma_start(b_t[:], bvec[:])
    ones_t = const.tile([1, 128], BF16, tag="ones")
    nc.vector.memset(ones_t[:], 1.0)
    # Cast x_self on gpsimd (otherwise idle): a DVE-resident cast would
    # sit ahead of every fold in the DVE program and stall them all until
    # the xs dma lands.
    xs_bf = const.tile([128, NG * F], BF16, tag="xs_bf")
    nc.gpsimd.tensor_copy(xs_bf[:], xs_all[:])

    obuf = const.tile([128, NG * D], F32, tag="obuf")

    def issue_group(g):
        # Both tensors of a group share ONE tile: [t0 rows | t1 rows], so
        # the fold levels fuse into single DVE ops over a [p, 2, x] view.
        t = xpool.tile([128, 2 * GF], F32, tag="t01")
        if g < NG - 1:
            nc.sync.dma_start(t[:, 0:GF], xn0[:, g * GF:(g + 1) * GF])
            nc.sync.dma_start(t[:, GF:2 * GF], xn1[:, g * GF:(g + 1) * GF])
        else:
            # Last group loads as halves so the fold overlaps the final
            # DMAs, shrinking the pipeline-drain tail.
            nc.sync.dma_start(t[:, 0:HG], xn0[:, g * GF:g * GF + HG])
            nc.sync.dma_start(t[:, GF:GF + HG], xn1[:, g * GF:g * GF + HG])
            nc.sync.dma_start(t[:, HG:GF], xn0[:, g * GF + HG:(g + 1) * GF])
            nc.sync.dma_start(t[:, GF + HG:2 * GF],
                              xn1[:, g * GF + HG:(g + 1) * GF])
        return t

    def fold_fused(t):
        """Fold both tensors' 32 slices with ONE DVE op per level, using
        [p, 2, x] strided views over the shared tile (same cycles as two
        ops -- DVE cost is max AP free size -- but half the instruction
        count and per-op overhead).  Sums land at fb[:, 0:F] (xn0) and
        fb[:, 16F:16F+F] (xn1)."""
        fb = fpool.tile([128, 2 * 16 * F], BF16, tag="fb")
        tv = t[:].rearrange("p (t x) -> p t x", t=2)
        fv = fb[:].rearrange("p (t x) -> p t x", t=2)
        nc.vector.tensor_add(fv[:, :, :], tv[:, :, 0:16 * F],
                             tv[:, :, 16 * F:32 * F])
        for lv in (8, 4, 2, 1):
            nc.vector.tensor_add(fv[:, :, 0:lv * F], fv[:, :, 0:lv * F],
                                 fv[:, :, lv * F:2 * lv * F])
        return fb

    def fold_split(t):
        """Last-group reduction: per-tensor, half-at-a-time, each half
        folding its 16 slices pairwise as soon as its dma lands (the
        fused form would wait for the final half of BOTH tensors)."""
        fb = fpool.tile([128, 2 * 16 * F], BF16, tag="fb")
        for off in (0, GF):
            fo = off * 16 * F // GF  # 0 or 16*F: this tensor's fb half
            nc.vector.tensor_add(fb[:, fo:fo + 8 * F], t[:, off:off + 8 * F],
                                 t[:, off + 8 * F:off + 16 * F])
            nc.vector.tensor_add(fb[:, fo + 8 * F:fo + 16 * F],
                                 t[:, off + HG:off + HG + 8 * F],
                                 t[:, off + HG + 8 * F:off + HG + 16 * F])
            nc.vector.tensor_add(fb[:, fo:fo + 8 * F], fb[:, fo:fo + 8 * F],
                                 fb[:, fo + 8 * F:fo + 16 * F])
            for lv in (4, 2, 1):
                nc.vector.tensor_add(fb[:, fo:fo + lv * F],
                                     fb[:, fo:fo + lv * F],
                                     fb[:, fo + lv * F:fo + 2 * lv * F])
        return fb

    def self_part(g):
        """x_self projection for group g: independent of the neighbour
        stream, so it runs ahead and stays off the drain-tail chain."""
        pc = ppool.tile([128, 128], F32, tag="pacc_s")
        nc.tensor.matmul(pc[:], xs_bf[:, g * F:(g + 1) * F], ident,
                         start=True, stop=True)
        sx = spool.tile([128, 128], BF16, tag="sacc_s")
        nc.scalar.activation(sx[:], pc[:], mybir.ActivationFunctionType.Copy)
        po = pout.tile([128, D], F32, tag="po")
        nc.tensor.matmul(po[:, 0:HALF], sx[:], wS_t, start=True, stop=False)
        nc.tensor.matmul(po[:, 0:HALF], ones_t[:], b_t[:, 0:HALF],
                         start=False, stop=True)
        return po

    def compute_group(g, po, fb):
        # The xn0 transpose/copy/w0-matmul chain runs as soon as fb's xn0
        # half is folded -- it must not wait for xn1's (later) fold, which
        # matters on the drain tail where xn1's last half lands last.
        pacc = ppool.tile([128, 2 * 128], F32, tag="pacc")
        sacc = spool.tile([128, 2 * 128], BF16, tag="sacc")
        nc.tensor.matmul(pacc[:, 0:128], fb[:, 0:F], ident,
                         start=True, stop=True)
        nc.scalar.activation(sacc[:, 0:128], pacc[:, 0:128],
                             mybir.ActivationFunctionType.Copy)
        nc.tensor.matmul(po[:, HALF:D], sacc[:, 0:128], w0_t,
                         start=True, stop=False)
        nc.tensor.matmul(pacc[:, 128:256], fb[:, 16 * F:16 * F + F], ident,
                         start=True, stop=True)
        nc.scalar.activation(sacc[:, 128:256], pacc[:, 128:256],
                             mybir.ActivationFunctionType.Copy)
        nc.tensor.matmul(po[:, HALF:D], sacc[:, 128:256], w1_t,
                         start=False, stop=False)
        nc.tensor.matmul(po[:, HALF:D], ones_t[:], b_t[:, HALF:D],
                         start=False, stop=True)

        nc.scalar.activation(obuf[:, g * D:(g + 1) * D], po[:],
                             mybir.ActivationFunctionType.Relu)
        # Batched stores on the otherwise-idle scalar ring, enqueued right
        # after the relu that gates them so they fire mid-stream.
        if g == 4:
            nc.scalar.dma_start(out_d[:, 0:5 * D], obuf[:, 0:5 * D])
        elif g == 8:
            nc.scalar.dma_start(out_d[:, 5 * D:9 * D], obuf[:, 5 * D:9 * D])
        elif g == 9:
            nc.scalar.dma_start(out_d[:, 9 * D:NG * D], obuf[:, 9 * D:NG * D])

    pending = [issue_group(g) for g in range(PRE)]
    for g in range(NG):
        t = pending.pop(0)
        if g + PRE < NG:
            pending.append(issue_group(g + PRE))
        po = self_part(g)
        fb = fold_fused(t) if g < NG - 1 else fold_split(t)
        compute_group(g, po, fb)


def build_nc():
    nc = bacc.Bacc("TRN2", target_bir_lowering=False, debug=False)
    # bh rows are partition-major: dram row index = 10*p + g.
    xn0 = nc.dram_tensor("xn0", [128, NG * GF], F32, kind="ExternalInput")
    xn1 = nc.dram_tensor("xn1", [128, NG * GF], F32, kind="ExternalInput")
    xs = nc.dram_tensor("xs", [128, NG * F], F32, kind="ExternalInput")
    wpack = nc.dram_tensor("wpack", [128, 4 * 128], BF16, kind="ExternalInput")
    bvec = nc.dram_tensor("bvec", [1, D], BF16, kind="ExternalInput")
    out = nc.dram_tensor("out", [128, NG * D], F32, kind="ExternalOutput")

    ins = [t.ap() for t in (xn0, xn1, xs, wpack, bvec)]
    with tile.TileContext(nc) as tc:
        _tile_kernel(tc, [out.ap()], ins)
    nc.compile()
    return nc


def make_in_maps(x_self, x_neigh_0, x_neigh_1, w_self, w_neigh_0, w_neigh_1, b):
    """Shard full inputs into per-core input maps (batch axis, 8 ways)."""
    x_self = np.ascontiguousarray(np.asarray(x_self, dtype=np.float32))
    x_neigh_0 = np.ascontiguousarray(np.asarray(x_neigh_0, dtype=np.float32))
    x_neigh_1 = np.ascontiguousarray(np.asarray(x_neigh_1, dtype=np.float32))
    scale = np.float32(1.0 / (N * NR))
    ident = np.eye(128, dtype=np.float32)
    wpack = np.concatenate([
        ident,
        np.asarray(w_self, dtype=np.float32),
        np.asarray(w_neigh_0, dtype=np.float32) * scale,
        np.asarray(w_neigh_1, dtype=np.float32) * scale,
    ], axis=1).astype(NPBF16)
    bvec = np.asarray(b, dtype=np.float32).reshape(1, D).astype(NPBF16)

    in_maps = []
    for c in range(NCORES):
        bs = slice(c * BSH, (c + 1) * BSH)
        in_maps.append({
            "xn0": np.ascontiguousarray(x_neigh_0[bs].reshape(128, NG * GF)),
            "xn1": np.ascontiguousarray(x_neigh_1[bs].reshape(128, NG * GF)),
            "xs": np.ascontiguousarray(x_self[bs].reshape(128, NG * F)),
            "wpack": np.ascontiguousarray(wpack), "bvec": bvec,
        })
    return in_maps


_NC_CACHE = None


def kernel(x_self, x_neigh_0, x_neigh_1, w_self, w_neigh_0, w_neigh_1, b):
    global _NC_CACHE
    if _NC_CACHE is None:
        _NC_CACHE = build_nc()
    in_maps = make_in_maps(x_self, x_neigh_0, x_neigh_1,
                           w_self, w_neigh_0, w_neigh_1, b)
    res = bass_utils.run_bass_kernel_spmd(
        _NC_CACHE, in_maps, core_ids=list(range(NCORES)))
    out = np.concatenate([r["out"].reshape(BH, D) for r in res.results], axis=0)
    return out.reshape(B, H, D)


# revision 22
# speedup vs baseline: 1.0283x; 1.0206x over previous
"""Trainium2 Bass kernel: MeanHinAggregator (GNN message passing).

Reference computation (per batch-head element bh):
    z_r  = mean_n(x_neigh_r[bh, n, :]) @ w_neigh_r          (r = 0, 1)
    out  = relu(concat(x_self[bh] @ w_self, (z0 + z1) / 2) + b)

Strategy (pure data parallel over 8 NeuronCores, batch axis sharded):
  * Per core: B_shard=128, H=10 -> 1280 bh rows, mapped partition-major
    (bh = 10*p + g, p = SBUF partition, g = group 0..9) so per-partition
    DRAM runs are contiguous across adjacent groups: x_self loads in ONE
    dma (5 KiB/partition) and the output stores in three batched dmas.
  * The binding constraint is AGGREGATE HBM bandwidth: 8 cores x 44 MB of
    fp32 input against a ~3.25 TB/s ceiling.  A single core streams at
    ~430 GB/s; 8x430 oversubscribes the ceiling, so the arbiter squeezes
    1-2 unlucky cores per run and those define the max-across-cores time
    (self-pacing below fair share was tried and does NOT prevent this; it
    only adds time).  Even groups load whole bh rows (16 KiB packets);
    the last group loads as two halves (8 KiB packets) so the fold can
    start on the first half, shortening the pipeline-drain tail.
  * All neighbour loads ride ONE ring (sync), strictly alternating
    xn0/xn1: DMA engines drain whole DMAs in global enqueue order, so
    in-FIFO alternation is the only way to guarantee the two tensors
    stream together.  Consts (packed into a single dma) + x_self are
    enqueued at t~0 on the scalar ring so they are served inside the
    first neighbour DMAs; stores also ride the scalar ring, gated by
    relu semaphores.
  * Mean over 32 neighbours: in-place strided adds on the Vector engine.
    The first add reads fp32 and writes bf16; the rest run in bf16 which
    gets the DVE 2x packed-16-bit mode.  The x_self projection runs ahead
    of the neighbour stream (it has no neighbour dependency), keeping the
    drain-tail chain to: last half-fold -> transpose -> copy -> two
    matmuls -> relu -> small store.
  * The folded [bh, f] slices are transposed into [f, bh] PSUM layout with
    single-pass bf16 matmuls against a bf16 identity.  Projection matmuls,
    also bf16, use host-precast weights with the 1/(N*NR) mean scale
    folded in.  Bias is added with a K=1 matmul; final ReLU emits fp32.
"""

import numpy as np
import ml_dtypes

import concourse.bacc as bacc
import concourse.bass as bass
import concourse.tile as tile
from concourse import bass_utils, mybir
from concourse._compat import with_exitstack

B, H, N, F = 1024, 10, 32, 128
HALF = 128
D = 2 * HALF
NR = 2
NCORES = 8
BSH = B // NCORES        # 128 batch rows per core
BH = BSH * H             # 1280 bh rows per core
NG = 10                  # groups per core (128 bh rows each)
GF = N * F               # 4096 elements per bh row
HG = GF // 2
F32 = mybir.dt.float32
BF16 = mybir.dt.bfloat16
NPBF16 = ml_dtypes.bfloat16
PRE = 4                  # groups of neighbour tiles in flight


@with_exitstack
def _tile_kernel(ctx, tc, outs, ins):
    nc = tc.nc
    xn0, xn1, xs, wpack, bvec = ins
    (out_d,) = outs

    const = ctx.enter_context(tc.tile_pool(name="const", bufs=1))
    xpool = ctx.enter_context(tc.tile_pool(name="xp", bufs=PRE))
    fpool = ctx.enter_context(tc.tile_pool(name="fp", bufs=3))
    spool = ctx.enter_context(tc.tile_pool(name="sp", bufs=3))
    ppool = ctx.enter_context(tc.tile_pool(name="ps", bufs=2, space="PSUM"))
    pout = ctx.enter_context(tc.tile_pool(name="po", bufs=2, space="PSUM"))

    # Consts + x_self ride the scalar ring, enqueued by the ACT engine's
    # first instructions (t~0) so they are served within the first one or
    # two neighbour DMAs of the sync ring.  This keeps the neighbour
    # stream's first packet off the critical path of the const loads.
    # ident/wS/w0/w1 arrive packed in one [128, 512] tensor (single dma);
    # they are not needed until the first transpose matmul (~13us in).
    xs_all = const.tile([128, NG * F], F32, tag="xs_all")
    nc.scalar.dma_start(xs_all[:], xs[:])
    wp_t = const.tile([128, 4 * 128], BF16, tag="wp")
    nc.scalar.dma_start(wp_t[:], wpack[:])
    ident = wp_t[:, 0:128]
    wS_t = wp_t[:, 128:256]
    w0_t = wp_t[:, 256:384]
    w1_t = wp_t[:, 384:512]
    b_t = const.tile([1, D], BF16, tag="b")
    nc.scalar.dma_start(b_t[:], bvec[:])
    ones_t = const.tile([1, 128], BF16, tag="ones")
    nc.vector.memset(ones_t[:], 1.0)
    # Cast x_self on gpsimd (otherwise idle): a DVE-resident cast would
    # sit ahead of every fold in the DVE program and stall them all until
    # the xs dma lands.
    xs_bf = const.tile([128, NG * F], BF16, tag="xs_bf")
    nc.gpsimd.tensor_copy(xs_bf[:], xs_all[:])

    obuf = const.tile([128, NG * D], F32, tag="obuf")

    def issue_group(g):
        # Both tensors of a group share ONE tile: [t0 rows | t1 rows], so
        # the fold levels fuse into single DVE ops over a [p, 2, x] view.
        t = xpool.tile([128, 2 * GF], F32, tag="t01")
        if g < NG - 2:
            nc.sync.dma_start(t[:, 0:GF], xn0[:, g * GF:(g + 1) * GF])
            nc.sync.dma_start(t[:, GF:2 * GF], xn1[:, g * GF:(g + 1) * GF])
        else:
            # Last two groups load as halves so their folds interleave
            # with the final DMAs instead of piling up after them (the
            # fused fold waits for BOTH tensors and runs 5us serial).
            nc.sync.dma_start(t[:, 0:HG], xn0[:, g * GF:g * GF + HG])
            nc.sync.dma_start(t[:, GF:GF + HG], xn1[:, g * GF:g * GF + HG])
            nc.sync.dma_start(t[:, HG:GF], xn0[:, g * GF + HG:(g + 1) * GF])
            nc.sync.dma_start(t[:, GF + HG:2 * GF],
                              xn1[:, g * GF + HG:(g + 1) * GF])
        return t

    def fold_fused(t):
        """Fold both tensors' 32 slices with ONE DVE op per level, using
        [p, 2, x] strided views over the shared tile (same cycles as two
        ops -- DVE cost is max AP free size -- but half the instruction
        count and per-op overhead).  Sums land at fb[:, 0:F] (xn0) and
        fb[:, 16F:16F+F] (xn1)."""
        fb = fpool.tile([128, 2 * 16 * F], BF16, tag="fb")
        tv = t[:].rearrange("p (t x) -> p t x", t=2)
        fv = fb[:].rearrange("p (t x) -> p t x", t=2)
        nc.vector.tensor_add(fv[:, :, :], tv[:, :, 0:16 * F],
                             tv[:, :, 16 * F:32 * F])
        for lv in (8, 4, 2, 1):
            nc.vector.tensor_add(fv[:, :, 0:lv * F], fv[:, :, 0:lv * F],
                                 fv[:, :, lv * F:2 * lv * F])
        return fb

    def fold_split(t):
        """Last-group reduction: per-tensor, half-at-a-time, each half
        folding its 16 slices pairwise as soon as its dma lands (the
        fused form would wait for the final half of BOTH tensors)."""
        fb = fpool.tile([128, 2 * 16 * F], BF16, tag="fb")
        for off in (0, GF):
            fo = off * 16 * F // GF  # 0 or 16*F: this tensor's fb half
            nc.vector.tensor_add(fb[:, fo:fo + 8 * F], t[:, off:off + 8 * F],
                                 t[:, off + 8 * F:off + 16 * F])
            nc.vector.tensor_add(fb[:, fo + 8 * F:fo + 16 * F],
                                 t[:, off + HG:off + HG + 8 * F],
                                 t[:, off + HG + 8 * F:off + HG + 16 * F])
            nc.vector.tensor_add(fb[:, fo:fo + 8 * F], fb[:, fo:fo + 8 * F],
                                 fb[:, fo + 8 * F:fo + 16 * F])
            for lv in (4, 2, 1):
                nc.vector.tensor_add(fb[:, fo:fo + lv * F],
                                     fb[:, fo:fo + lv * F],
                                     fb[:, fo + lv * F:fo + 2 * lv * F])
        return fb

    def self_part(g):
        """x_self projection for group g: independent of the neighbour
        stream, so it runs ahead and stays off the drain-tail chain."""
        pc = ppool.tile([128, 128], F32, tag="pacc_s")
        nc.tensor.matmul(pc[:], xs_bf[:, g * F:(g + 1) * F], ident,
                         start=True, stop=True)
        sx = spool.tile([128, 128], BF16, tag="sacc_s")
        nc.scalar.activation(sx[:], pc[:], mybir.ActivationFunctionType.Copy)
        po = pout.tile([128, D], F32, tag="po")
        nc.tensor.matmul(po[:, 0:HALF], sx[:], wS_t, start=True, stop=False)
        nc.tensor.matmul(po[:, 0:HALF], ones_t[:], b_t[:, 0:HALF],
                         start=False, stop=True)
        return po

    def compute_group(g, po, fb):
        # The xn0 transpose/copy/w0-matmul chain runs as soon as fb's xn0
        # half is folded -- it must not wait for xn1's (later) fold, which
        # matters on the drain tail where xn1's last half lands last.
        pacc = ppool.tile([128, 2 * 128], F32, tag="pacc")
        sacc = spool.tile([128, 2 * 128], BF16, tag="sacc")
        nc.tensor.matmul(pacc[:, 0:128], fb[:, 0:F], ident,
                         start=True, stop=True)
        nc.scalar.activation(sacc[:, 0:128], pacc[:, 0:128],
                             mybir.ActivationFunctionType.Copy)
        nc.tensor.matmul(po[:, HALF:D], sacc[:, 0:128], w0_t,
                         start=True, stop=False)
        nc.tensor.matmul(pacc[:, 128:256], fb[:, 16 * F:16 * F + F], ident,
                         start=True, stop=True)
        nc.scalar.activation(sacc[:, 128:256], pacc[:, 128:256],
                             mybir.ActivationFunctionType.Copy)
        nc.tensor.matmul(po[:, HALF:D], sacc[:, 128:256], w1_t,
                         start=False, stop=False)
        nc.tensor.matmul(po[:, HALF:D], ones_t[:], b_t[:, HALF:D],
                         start=False, stop=True)

        nc.scalar.activation(obuf[:, g * D:(g + 1) * D], po[:],
                             mybir.ActivationFunctionType.Relu)
        # Batched stores on the otherwise-idle scalar ring, enqueued right
        # after the relu that gates them so they fire mid-stream.
        if g == 4:
            nc.scalar.dma_start(out_d[:, 0:5 * D], obuf[:, 0:5 * D])
        elif g == 8:
            nc.scalar.dma_start(out_d[:, 5 * D:9 * D], obuf[:, 5 * D:9 * D])
        elif g == 9:
            nc.scalar.dma_start(out_d[:, 9 * D:NG * D], obuf[:, 9 * D:NG * D])

    pending = [issue_group(g) for g in range(PRE)]
    for g in range(NG):
        t = pending.pop(0)
        if g + PRE < NG:
            pending.append(issue_group(g + PRE))
        po = self_part(g)
        fb = fold_fused(t) if g < NG - 2 else fold_split(t)
        compute_group(g, po, fb)


def build_nc():
    nc = bacc.Bacc("TRN2", target_bir_lowering=False, debug=False)
    # bh rows are partition-major: dram row index = 10*p + g.
    xn0 = nc.dram_tensor("xn0", [128, NG * GF], F32, kind="ExternalInput")
    xn1 = nc.dram_tensor("xn1", [128, NG * GF], F32, kind="ExternalInput")
    xs = nc.dram_tensor("xs", [128, NG * F], F32, kind="ExternalInput")
    wpack = nc.dram_tensor("wpack", [128, 4 * 128], BF16, kind="ExternalInput")
    bvec = nc.dram_tensor("bvec", [1, D], BF16, kind="ExternalInput")
    out = nc.dram_tensor("out", [128, NG * D], F32, kind="ExternalOutput")

    ins = [t.ap() for t in (xn0, xn1, xs, wpack, bvec)]
    with tile.TileContext(nc) as tc:
        _tile_kernel(tc, [out.ap()], ins)
    nc.compile()
    return nc


def make_in_maps(x_self, x_neigh_0, x_neigh_1, w_self, w_neigh_0, w_neigh_1, b):
    """Shard full inputs into per-core input maps (batch axis, 8 ways)."""
    x_self = np.ascontiguousarray(np.asarray(x_self, dtype=np.float32))
    x_neigh_0 = np.ascontiguousarray(np.asarray(x_neigh_0, dtype=np.float32))
    x_neigh_1 = np.ascontiguousarray(np.asarray(x_neigh_1, dtype=np.float32))
    scale = np.float32(1.0 / (N * NR))
    ident = np.eye(128, dtype=np.float32)
    wpack = np.concatenate([
        ident,
        np.asarray(w_self, dtype=np.float32),
        np.asarray(w_neigh_0, dtype=np.float32) * scale,
        np.asarray(w_neigh_1, dtype=np.float32) * scale,
    ], axis=1).astype(NPBF16)
    bvec = np.asarray(b, dtype=np.float32).reshape(1, D).astype(NPBF16)

    in_maps = []
    for c in range(NCORES):
        bs = slice(c * BSH, (c + 1) * BSH)
        in_maps.append({
            "xn0": np.ascontiguousarray(x_neigh_0[bs].reshape(128, NG * GF)),
            "xn1": np.ascontiguousarray(x_neigh_1[bs].reshape(128, NG * GF)),
            "xs": np.ascontiguousarray(x_self[bs].reshape(128, NG * F)),
            "wpack": np.ascontiguousarray(wpack), "bvec": bvec,
        })
    return in_maps


_NC_CACHE = None


def kernel(x_self, x_neigh_0, x_neigh_1, w_self, w_neigh_0, w_neigh_1, b):
    global _NC_CACHE
    if _NC_CACHE is None:
        _NC_CACHE = build_nc()
    in_maps = make_in_maps(x_self, x_neigh_0, x_neigh_1,
                           w_self, w_neigh_0, w_neigh_1, b)
    res = bass_utils.run_bass_kernel_spmd(
        _NC_CACHE, in_maps, core_ids=list(range(NCORES)))
    out = np.concatenate([r["out"].reshape(BH, D) for r in res.results], axis=0)
    return out.reshape(B, H, D)
